# Initial kernel scaffold
#
"""CrossHeadProjectionV2 Trainium2 kernel.

out[b,n,t,s] = sum_m x[m,t,s] * (W_t + C_s)[m,n]
  W_t = (w + I) + qw1[t]^T qw2[t] + diag(qdd[t])   (host-folded, per-t 16x16)
  C_s = kw1[s]^T kw2[s] + diag(kdd[s])             (per-s 16x16, rank-2 + diag)

Shard T over 8 cores (256 t each). Tiles of 8 t's x 16 heads = 128
partitions, S=2048 free; pipeline unit = half tile (1024 cols = 2 psum
chunks). Per half:
  PE:  po        = Wbd^T x            (block-diag 16x16 per t, 2 mms)
       ph[i]     = Rep^T (x*kw1r_i)   (group-sum + n-broadcast, 4 mms)
       po       += Id^T u_i, Id^T tmp2  (6 mms)
  DVE: tmp_i = x*kw1r_i ; u_i = h_i*kw2r_i   (bf16 2x TT)
  GPS: tmp2 = x*kddr
  ACT/DVE: psum evacuations (h -> sbuf bf16, out -> sbuf f32)
Software-pipelined: stage B (idents/out-copy/dma-out) of half k-1 is
emitted after stage A of half k so every engine FIFO has ready work.
"""

import numpy as np
import ml_dtypes

bf = ml_dtypes.bfloat16

B, N, T, S = 1, 16, 2048, 2048
G, I, M = 1, 2, 16
NCORES = 8
TC = T // NCORES        # 256 t per core
TB = 8                  # t rows per tile (TB*M = 128 partitions)
SC = 512                # psum chunk (one fp32 bank)
HW = 2 * SC             # half-tile width

# engine assignment knobs
TMP2_ENGINE = "vector"   # GPSIMD shares DVE's SBUF port: keep Pool idle
OUTCOPY_ENGINES = ("scalar", "scalar")
OUTCOPY_VECTOR_EVERY = 0  # >0: chunk-1 po copy on DVE every Nth half
PO_MERGED = False     # po as one [128,2,SC] tile, single out-copy
BUFS = dict(xp=6, tmpp=8, t2p=8, hp=8, up=8, op=8, ps_o=4, ps_h=2)
TMP_LOOKAHEAD = 0
U_ROUTE = "act2"      # ph evac on ACT (own port), DVE muls at 2x
OUT_BF16 = True
TMP1_POOL_EVERY = 0   # >0: emit tmp1 on gpsimd every Nth half-iteration
REPS_FIRST = True     # emit rep matmuls before mains (unblocks DVE/ACT chain)
B_BEFORE_U = False    # emit stage_b(it-1) before stage_u(it)
TMP_FUSED = False     # tmp0+tmp1 as one broadcast-x DVE op
TMP_TILEWIDE = False  # tmp/tmp2 muls span the whole tile (fewer DVE ops)
TMP_AFTER_U = False   # emit stage_tmp after stage_u (needs TMP_LOOKAHEAD>0)
U_SPLIT = False       # per-i h-copies/u_c1 (shorter chain into u_c1)
UC1_AFTER_TMP = True  # scheduling edge: next tmps before u_c1 on DVE
U_C1_POOL_EVERY = 0   # >0: u_c1 on gpsimd every Nth half-iteration
U_DIRECT_EVERY = 0    # >0: chunk-0 u-mul direct from psum every Nth half

# Diagnostic amputations (WRONG RESULTS - timing only)
DIAG_NO_IDENTS = False   # skip u/tmp2 accumulation matmuls
DIAG_NO_UMUL = False     # skip u muls + hsb copies (idents read stale u)
DIAG_NO_TMP2 = False     # skip tmp2 mul (gpsimd idle)
DIAG_NO_REPS = False     # skip rep matmuls + tmp muls + u chain entirely

_cache = {}


def _build(tc_size=TC, reps=1, inner=1):
    import contextlib
    import concourse.mybir as mybir
    import concourse.tile as tile
    from concourse import bacc

    bf16, f32 = mybir.dt.bfloat16, mybir.dt.float32
    nt = tc_size // TB
    nh = nt * 2

    nc = bacc.Bacc("TRN2", target_bir_lowering=False, debug=False)

    x_d = nc.dram_tensor("x", [M, tc_size, S], bf16, kind="ExternalInput")
    wbd_d = nc.dram_tensor("wbd", [nt, 128, 128], bf16, kind="ExternalInput")
    rep_d = nc.dram_tensor("rep", [128, 128], bf16, kind="ExternalInput")
    idn_d = nc.dram_tensor("idn", [128, 128], bf16, kind="ExternalInput")
    ewin_d = nc.dram_tensor("ewin", [128, 3, S], bf16, kind="ExternalInput")
    ewout_d = nc.dram_tensor("ewout", [128, 2, S], bf16, kind="ExternalInput")
    out_dt = bf16 if OUT_BF16 else f32
    out_d = nc.dram_tensor("out", [M, tc_size, S], out_dt,
                           kind="ExternalOutput")

    def eng(name):
        return getattr(nc, {"scalar": "scalar", "vector": "vector",
                            "gpsimd": "gpsimd"}[name])

    def copy_on(engine_name, dst, src):
        if engine_name == "scalar":
            nc.scalar.copy(dst, src)
        else:
            nc.vector.tensor_copy(dst, src)

    with tile.TileContext(nc) as tc:
        with (
            tc.tile_pool(name="const", bufs=1) as constp,
            tc.tile_pool(name="xp", bufs=BUFS["xp"]) as xp,
            tc.tile_pool(name="tmpp", bufs=BUFS["tmpp"]) as tmpp,
            tc.tile_pool(name="t2p", bufs=BUFS["t2p"]) as t2p,
            tc.tile_pool(name="hp", bufs=BUFS["hp"]) as hp,
            tc.tile_pool(name="up", bufs=BUFS["up"]) as up,
            tc.tile_pool(name="op", bufs=BUFS["op"]) as op,
            tc.tile_pool(name="ps_o", bufs=BUFS["ps_o"], space="PSUM") as ps_o,
            tc.tile_pool(name="ps_h", bufs=BUFS["ps_h"], space="PSUM") as ps_h,
        ):
            # Loads ordered by first use: ewin[:, 0:2] feeds the first tmp
            # ops, then the x tiles (emitted by stage_tmp(0)); everything
            # else arrives behind them so the pipeline ramps fast.
            ewin = constp.tile([128, 3, S], bf16)
            nc.sync.dma_start(ewin[:, 0:2], ewin_d.ap()[:, 0:2])
            rep = constp.tile([128, 128], bf16)
            wbd = constp.tile([128, nt, 128], bf16)
            idn = constp.tile([128, 128], bf16)
            ewout = constp.tile([128, 2, S], bf16)

            def load_late_consts():
                nc.sync.dma_start(rep[:], rep_d.ap())
                nc.sync.dma_start(ewin[:, 2:3], ewin_d.ap()[:, 2:3])
                nc.sync.dma_start(ewout[:], ewout_d.ap())
                nc.sync.dma_start(idn[:], idn_d.ap())
                nc.sync.dma_start(
                    wbd[:], wbd_d.ap().rearrange("t p q -> p t q"))

            xt_of_tile = {}
            tw_tiles = {}  # tile -> (tmp, tmp2) for TMP_TILEWIDE
            tmps = {}   # half -> (tmp, tmp2), produced one iteration early
            state = {}  # half -> dict for stage B
            late_consts_done = []
            pending_uc1 = []  # last u_c1 instr, to order next tmps ahead

            def load_x(ti):
                if ti in xt_of_tile or ti >= nt:
                    return
                xt = xp.tile([128, S], bf16)
                src = x_d.ap()[:, ti * TB:(ti + 1) * TB, :].rearrange(
                    "m tb s -> tb m s"
                )
                nc.sync.dma_start(xt[:], src)
                xt_of_tile[ti] = xt

            def stage_tmp(h):
                """Elementwise pre-multiplies for half h (emitted one
                iteration ahead so DVE never waits on this iteration's PE)."""
                ti, half = divmod(h, 2)
                hs = slice(half * HW, (half + 1) * HW)
                load_x(ti)
                load_x(ti + 1)
                load_x(ti + 2)
                xt = xt_of_tile[ti]
                if TMP_TILEWIDE:
                    if half == 1:
                        tmp, tmp2 = tw_tiles[ti]
                        tmps[h] = dict(tmp=tmp, tmp2=tmp2, off=HW)
                        return
                    tmp = tmpp.tile([128, 2, S], bf16)
                    tmp2 = t2p.tile([128, S], bf16)
                    tmp_instrs = [
                        nc.vector.tensor_mul(tmp[:, 0], xt[:], ewin[:, 0]),
                        nc.vector.tensor_mul(tmp[:, 1], xt[:], ewin[:, 1]),
                    ]
                    if UC1_AFTER_TMP and pending_uc1:
                        from concourse.tile import add_dep_helper
                        uc1 = pending_uc1.pop()
                        for tin in tmp_instrs:
                            add_dep_helper(uc1.ins, tin.ins, sync=False,
                                           reason="order next tmps before u_c1")
                    if not late_consts_done:
                        late_consts_done.append(True)
                        load_late_consts()
                    eng(TMP2_ENGINE).tensor_mul(tmp2[:], xt[:], ewin[:, 2])
                    tw_tiles[ti] = (tmp, tmp2)
                    tmps[h] = dict(tmp=tmp, tmp2=tmp2, off=0)
                    return
                tmp = tmpp.tile([128, 2, HW], bf16)
                tmp_instrs = []
                if DIAG_NO_REPS:
                    if not late_consts_done:
                        late_consts_done.append(True)
                        load_late_consts()
                    nc.gpsimd.memset(tmp[:, :, 0:8], 0.0)
                    tmp2 = t2p.tile([128, HW], bf16)
                    if DIAG_NO_TMP2:
                        nc.gpsimd.memset(tmp2[:, 0:8], 0.0)
                    else:
                        eng(TMP2_ENGINE).tensor_mul(tmp2[:], xt[:, hs],
                                                    ewin[:, 2, hs])
                    tmps[h] = dict(tmp=tmp, tmp2=tmp2, off=0)
                    return
                if TMP_FUSED:
                    xb = xt[:, hs].rearrange(
                        "p (one s) -> p one s", one=1
                    ).broadcast_to((128, 2, HW))
                    nc.vector.tensor_mul(tmp[:], xb, ewin[:, 0:2, hs])
                else:
                    tmp_instrs.append(nc.vector.tensor_mul(
                        tmp[:, 0], xt[:, hs], ewin[:, 0, hs]))
                    tmp1_eng = nc.vector
                    if TMP1_POOL_EVERY and h % TMP1_POOL_EVERY == 0:
                        tmp1_eng = nc.gpsimd
                    ins1 = tmp1_eng.tensor_mul(tmp[:, 1], xt[:, hs],
                                               ewin[:, 1, hs])
                    if tmp1_eng is nc.vector:
                        tmp_instrs.append(ins1)
                if UC1_AFTER_TMP and pending_uc1:
                    from concourse.tile import add_dep_helper
                    uc1 = pending_uc1.pop()
                    for tin in tmp_instrs:
                        add_dep_helper(uc1.ins, tin.ins, sync=False,
                                       reason="order next tmps before u_c1")
                if not late_consts_done:
                    late_consts_done.append(True)
                    load_late_consts()
                tmp2 = t2p.tile([128, HW], bf16)
                if DIAG_NO_TMP2:
                    nc.gpsimd.memset(tmp2[:, 0:8], 0.0)
                else:
                    eng(TMP2_ENGINE).tensor_mul(tmp2[:], xt[:, hs],
                                                ewin[:, 2, hs])
                tmps[h] = dict(tmp=tmp, tmp2=tmp2, off=0)

            def stage_mm(h):
                ti, half = divmod(h, 2)
                xt = xt_of_tile[ti]
                tdict = tmps.pop(h)
                tmp, tmp2, toff = tdict["tmp"], tdict["tmp2"], tdict["off"]

                main_stop = DIAG_NO_IDENTS

                def emit_mains():
                    if PO_MERGED:
                        po = ps_o.tile([128, 2, SC], f32)
                        po_cs = [po[:, c] for c in range(2)]
                        st_extra = {"po": po}
                    else:
                        po_cs, st_extra = [], {}
                    for c in range(2):
                        cs = slice(half * HW + c * SC,
                                   half * HW + (c + 1) * SC)
                        if PO_MERGED:
                            po_c = po_cs[c]
                        else:
                            po_c = ps_o.tile([128, SC], f32)
                            po_cs.append(po_c)
                        nc.tensor.matmul(po_c[:] if not PO_MERGED else po_c,
                                         wbd[:, ti], xt[:, cs],
                                         start=True, stop=main_stop)
                    return po_cs, st_extra

                def emit_reps():
                    if DIAG_NO_REPS:
                        return None
                    if U_ROUTE == "act4":
                        phs = []
                        for c in range(2):
                            for i in range(2):
                                ph_ci = ps_h.tile([128, SC], f32)
                                nc.tensor.matmul(
                                    ph_ci[:], rep[:],
                                    tmp[:, i, toff + c * SC:toff + (c + 1) * SC],
                                    start=True, stop=True)
                                phs.append(ph_ci)
                        return phs
                    if U_ROUTE == "merged":
                        ph = ps_h.tile([128, 2, 2, SC], f32)
                        for c in range(2):
                            for i in range(2):
                                nc.tensor.matmul(
                                    ph[:, c, i], rep[:],
                                    tmp[:, i, toff + c * SC:toff + (c + 1) * SC],
                                    start=True, stop=True)
                        return ph
                    phs = []
                    for c in range(2):
                        ph = ps_h.tile([128, 2, SC], f32)
                        for i in range(2):
                            nc.tensor.matmul(
                                ph[:, i], rep[:],
                                tmp[:, i, toff + c * SC:toff + (c + 1) * SC],
                                start=True, stop=True)
                        phs.append(ph)
                    return phs

                if REPS_FIRST:
                    phs = emit_reps()
                    po_cs, st_extra = emit_mains()
                else:
                    po_cs, st_extra = emit_mains()
                    phs = emit_reps()
                state[h] = dict(ti=ti, half=half, po_cs=po_cs, phs=phs,
                                tmp2=tmp2, toff=toff, **st_extra)

            def stage_u(h):
                st = state[h]
                half, phs = st["half"], st.pop("phs")
                u = up.tile([128, 2, HW], bf16)
                if DIAG_NO_UMUL or DIAG_NO_REPS:
                    nc.gpsimd.memset(u[:, :, 0:8], 0.0)
                    st["u"] = u
                    return
                hs = slice(half * HW, (half + 1) * HW)
                route = U_ROUTE
                if route == "alternate":
                    # balance DVE (1x psum TT) vs ACT (evac copy) load
                    route = "direct0" if h % 2 == 0 else "act"
                if route == "merged":
                    # single 4-bank ph tile; one ACT evac + one DVE mul
                    ph = phs
                    hsb = hp.tile([128, 2, 2, SC], bf16)
                    nc.scalar.copy(hsb[:], ph[:])
                    u_ap = u[:].rearrange("p i (c s) -> p c i s", c=2)
                    ew_ap = ewout[:, :, hs].rearrange(
                        "p i (c s) -> p c i s", c=2)
                    nc.vector.tensor_mul(u_ap, hsb[:], ew_ap)
                    st["u"] = u
                    return
                if route == "act4":
                    # per-(chunk,rank) 1-bank ph tiles: finest psum recycle
                    u_cs = []
                    for c in range(2):
                        u_c = up.tile([128, 2, SC], bf16)
                        for i in range(2):
                            cslc = slice(half * HW + c * SC,
                                         half * HW + (c + 1) * SC)
                            hsb_ci = hp.tile([128, SC], bf16)
                            nc.scalar.copy(hsb_ci[:], phs[2 * c + i][:])
                            nc.vector.tensor_mul(u_c[:, i], hsb_ci[:],
                                                 ewout[:, i, cslc])
                        u_cs.append(u_c)
                    st["u_cs"] = u_cs
                    return
                if route == "act2c":
                    # act2 with per-chunk u tiles (contiguous writes,
                    # earlier release into idents)
                    u_cs = []
                    for c in range(2):
                        cslc = slice(half * HW + c * SC, half * HW + (c + 1) * SC)
                        hsb = hp.tile([128, 2, SC], bf16)
                        nc.scalar.copy(hsb[:], phs[c][:])
                        u_c = up.tile([128, 2, SC], bf16)
                        nc.vector.tensor_mul(u_c[:], hsb[:], ewout[:, :, cslc])
                        u_cs.append(u_c)
                    st["u_cs"] = u_cs
                    return
                if route == "act2":
                    # per-chunk ACT evac + DVE 2x mul (GPS idle, ACT loaded)
                    for c in range(2):
                        cslc = slice(half * HW + c * SC, half * HW + (c + 1) * SC)
                        if (U_DIRECT_EVERY and c == 0
                                and h % U_DIRECT_EVERY == 0):
                            # balance: skip ACT evac, mul straight from psum
                            nc.vector.tensor_mul(
                                u[:, :, 0:SC], phs[0][:], ewout[:, :, cslc])
                            continue
                        hsb = hp.tile([128, 2, SC], bf16)
                        nc.scalar.copy(hsb[:], phs[c][:])
                        nc.vector.tensor_mul(
                            u[:, :, c * SC:(c + 1) * SC], hsb[:],
                            ewout[:, :, cslc])
                    st["u"] = u
                    return
                if route == "act1":
                    # per-chunk ACT evacs into ONE hsb tile; single DVE mul
                    hsb = hp.tile([128, 2, 2, SC], bf16)
                    for c in range(2):
                        nc.scalar.copy(hsb[:, c], phs[c][:])
                    u_ap = u[:].rearrange("p i (c s) -> p c i s", c=2)
                    ew_ap = ewout[:, :, hs].rearrange(
                        "p i (c s) -> p c i s", c=2)
                    nc.vector.tensor_mul(u_ap, hsb[:], ew_ap)
                    st["u"] = u
                    return
                if route == "direct0":
                    # chunk 0: multiply straight out of PSUM on DVE (1x);
                    # chunk 1: ACT-copy to SBUF bf16 then DVE TT at 2x.
                    c0 = slice(half * HW, half * HW + SC)
                    c1 = slice(half * HW + SC, half * HW + HW)
                    hsb = hp.tile([128, 2, SC], bf16)
                    if U_SPLIT:
                        # per-i granularity: the first copy starts one
                        # rep-matmul earlier, shortening the chain into u_c1
                        for i in range(2):
                            nc.scalar.copy(hsb[:, i], phs[1][:, i])
                        nc.vector.tensor_mul(u[:, :, 0:SC], phs[0][:],
                                             ewout[:, :, c0])
                        for i in range(2):
                            nc.vector.tensor_mul(u[:, i, SC:HW], hsb[:, i],
                                                 ewout[:, i, c1])
                    else:
                        nc.vector.tensor_mul(u[:, :, 0:SC], phs[0][:],
                                             ewout[:, :, c0])
                        nc.scalar.copy(hsb[:], phs[1][:])
                        uc1_eng = nc.vector
                        if U_C1_POOL_EVERY and h % U_C1_POOL_EVERY == 0:
                            uc1_eng = nc.gpsimd
                        uc1 = uc1_eng.tensor_mul(u[:, :, SC:HW], hsb[:],
                                                 ewout[:, :, c1])
                        if UC1_AFTER_TMP and uc1_eng is nc.vector:
                            pending_uc1.clear()
                            pending_uc1.append(uc1)
                else:  # "act": both chunks evacuated by ACT, one 2x TT
                    hsb = hp.tile([128, 2, HW], bf16)
                    for c in range(2):
                        nc.scalar.copy(hsb[:, :, c * SC:(c + 1) * SC],
                                       phs[c][:])
                    nc.vector.tensor_mul(u[:], hsb[:], ewout[:, :, hs])
                st["u"] = u

            def stage_b(h):
                st = state.pop(h)
                ti, half, po_cs, tmp2 = (st["ti"], st["half"], st["po_cs"],
                                         st["tmp2"])
                u, u_cs = st.get("u"), st.get("u_cs")
                toff = st["toff"]
                ot = op.tile([128, 2, SC], out_dt)
                for c in range(2):
                    csl = slice(c * SC, (c + 1) * SC)
                    po_c = po_cs[c] if PO_MERGED else po_cs[c][:]
                    if not DIAG_NO_IDENTS:
                        u0 = u_cs[c][:, 0] if u_cs is not None else u[:, 0, csl]
                        u1 = u_cs[c][:, 1] if u_cs is not None else u[:, 1, csl]
                        nc.tensor.matmul(po_c, idn[:], u0,
                                         start=False, stop=False)
                        nc.tensor.matmul(po_c, idn[:], u1,
                                         start=False, stop=False)
                        nc.tensor.matmul(po_c, idn[:],
                                         tmp2[:, toff + c * SC:
                                              toff + (c + 1) * SC],
                                         start=False, stop=True)
                    if not PO_MERGED:
                        ename = OUTCOPY_ENGINES[(2 * half + c) % 2]
                        if (OUTCOPY_VECTOR_EVERY and c == 1
                                and h % OUTCOPY_VECTOR_EVERY == 0):
                            ename = "vector"
                        copy_on(ename, ot[:, c], po_c)
                if PO_MERGED:
                    copy_on(OUTCOPY_ENGINES[half % 2], ot[:], st["po"][:])
                dst = out_d.ap()[
                    :, ti * TB:(ti + 1) * TB, half * HW:(half + 1) * HW
                ].rearrange("n tb s -> tb n s")
                nc.sync.dma_start(dst, ot[:].rearrange("p c s -> p (c s)"))

            loop_cm = (tc.For_i(0, reps, 1) if reps > 1
                       else contextlib.nullcontext())
            with loop_cm:
              for _inner in range(inner):
                xt_of_tile.clear()
                tw_tiles.clear()
                for it in range(TMP_LOOKAHEAD):
                    stage_tmp(it)
                for it in range(nh + 1):
                    if not TMP_AFTER_U and it + TMP_LOOKAHEAD < nh:
                        stage_tmp(it + TMP_LOOKAHEAD)
                    if it < nh:
                        stage_mm(it)
                        if not B_BEFORE_U:
                            stage_u(it)
                    if TMP_AFTER_U and it + TMP_LOOKAHEAD < nh:
                        stage_tmp(it + TMP_LOOKAHEAD)
                    if it >= 1:
                        stage_b(it - 1)
                    if B_BEFORE_U and it < nh:
                        stage_u(it)

    nc.compile()
    return nc


def _prep_weights(qw1, qw2, kw1, kw2, qdd, kdd, w, tc_size=TC, ncores=NCORES):
    """Host-side weight folding. Returns per-core wbd + shared tiles."""
    nt = tc_size // TB
    wi = w[0].astype(np.float64) + np.eye(M)
    qw1f, qw2f = qw1[0, :, 0].astype(np.float64), qw2[0, :, 0].astype(np.float64)
    # W_t[m,n] = wi + sum_i qw1[t,i,m] qw2[t,i,n] + diag(qdd[t])
    Wt = wi[None] + np.einsum("tim,tin->tmn", qw1f, qw2f)
    Wt[:, np.arange(M), np.arange(M)] += qdd[0, :, 0].astype(np.float64)
    Wt = Wt.astype(np.float32)

    wbds = []
    for c in range(ncores):
        Wc = Wt[c * tc_size:(c + 1) * tc_size].reshape(nt, TB, M, M)
        wbd = np.zeros((nt, 128, 128), dtype=bf)
        for tb in range(TB):
            wbd[:, tb * M:(tb + 1) * M, tb * M:(tb + 1) * M] = Wc[:, tb].astype(bf)
        wbds.append(wbd)

    rep = np.zeros((128, 128), dtype=bf)
    for tb in range(TB):
        rep[tb * M:(tb + 1) * M, tb * M:(tb + 1) * M] = 1.0
    idn = np.eye(128, dtype=np.float32).astype(bf)

    kw1f = kw1[0, :, 0]  # [S, I, M]
    kw2f = kw2[0, :, 0]
    kddf = kdd[0, :, 0]  # [S, M]
    ewin = np.empty((128, 3, S), dtype=bf)
    ewin[:, 0] = np.tile(kw1f[:, 0, :].T, (TB, 1)).astype(bf)
    ewin[:, 1] = np.tile(kw1f[:, 1, :].T, (TB, 1)).astype(bf)
    ewin[:, 2] = np.tile(kddf.T, (TB, 1)).astype(bf)
    ewout = np.empty((128, 2, S), dtype=bf)
    ewout[:, 0] = np.tile(kw2f[:, 0, :].T, (TB, 1)).astype(bf)
    ewout[:, 1] = np.tile(kw2f[:, 1, :].T, (TB, 1)).astype(bf)
    return wbds, rep, idn, ewin, ewout


def _make_in_maps(inputs, qw1, qw2, kw1, kw2, qdd, kdd, w,
                  tc_size=TC, ncores=NCORES):
    wbds, rep, idn, ewin, ewout = _prep_weights(
        qw1, qw2, kw1, kw2, qdd, kdd, w, tc_size, ncores
    )
    x = np.asarray(inputs)[0]  # [N, T, S] f32
    in_maps = []
    for c in range(ncores):
        xc = np.ascontiguousarray(
            x[:, c * tc_size:(c + 1) * tc_size, :]
        ).astype(bf)
        in_maps.append({
            "x": xc, "wbd": wbds[c], "rep": rep, "idn": idn,
            "ewin": ewin, "ewout": ewout,
        })
    return in_maps


def kernel(inputs, qw1, qw2, kw1, kw2, qdd, kdd, w, trace=False):
    from concourse import bass_utils

    inputs = np.asarray(inputs, dtype=np.float32)
    qw1, qw2 = np.asarray(qw1, np.float32), np.asarray(qw2, np.float32)
    kw1, kw2 = np.asarray(kw1, np.float32), np.asarray(kw2, np.float32)
    qdd, kdd = np.asarray(qdd, np.float32), np.asarray(kdd, np.float32)
    w = np.asarray(w, np.float32)

    if "nc" not in _cache:
        _cache["nc"] = _build()
    nc = _cache["nc"]

    in_maps = _make_in_maps(inputs, qw1, qw2, kw1, kw2, qdd, kdd, w)
    res = bass_utils.run_bass_kernel_spmd(
        nc, in_maps, core_ids=list(range(NCORES)), trace=trace
    )
    outs = [np.asarray(r["out"], dtype=np.float32) for r in res.results]
    out = np.concatenate(outs, axis=1)  # [N,T,S]
    _cache["last_results"] = res
    return out.reshape(B, N, T, S).astype(np.float32)



# revision 1
# speedup vs baseline: 1.0681x; 1.0681x over previous
"""CrossHeadProjectionV2 Trainium2 kernel.

out[b,n,t,s] = sum_m x[m,t,s] * (W_t + C_s)[m,n]
  W_t = (w + I) + qw1[t]^T qw2[t] + diag(qdd[t])   (host-folded, per-t 16x16)
  C_s = kw1[s]^T kw2[s] + diag(kdd[s])             (per-s 16x16, rank-2 + diag)

Shard T over 8 cores (256 t each). Tiles of 8 t's x 16 heads = 128
partitions, S=2048 free; pipeline unit = half tile (1024 cols = 2 psum
chunks). Per half:
  PE:  po        = Wbd^T x            (block-diag 16x16 per t, 2 mms)
       ph[i]     = Rep^T (x*kw1r_i)   (group-sum + n-broadcast, 4 mms)
       po       += Id^T u_i, Id^T tmp2  (6 mms)
  DVE: tmp_i = x*kw1r_i ; u_i = h_i*kw2r_i   (bf16 2x TT)
  GPS: tmp2 = x*kddr
  ACT/DVE: psum evacuations (h -> sbuf bf16, out -> sbuf f32)
Software-pipelined: stage B (idents/out-copy/dma-out) of half k-1 is
emitted after stage A of half k so every engine FIFO has ready work.
"""

import numpy as np
import ml_dtypes

bf = ml_dtypes.bfloat16

B, N, T, S = 1, 16, 2048, 2048
G, I, M = 1, 2, 16
NCORES = 8
TC = T // NCORES        # 256 t per core
TB = 8                  # t rows per tile (TB*M = 128 partitions)
SC = 512                # psum chunk (one fp32 bank)
HW = 2 * SC             # half-tile width

# engine assignment knobs
TMP2_ENGINE = "vector"   # GPSIMD shares DVE's SBUF port: keep Pool idle
OUTCOPY_ENGINES = ("scalar", "scalar")
OUTCOPY_VECTOR_EVERY = 0  # >0: chunk-1 po copy on DVE every Nth half
PO_MERGED = False     # po as one [128,2,SC] tile, single out-copy
BUFS = dict(xp=6, tmpp=8, t2p=8, hp=8, up=8, op=8, ps_o=4, ps_h=2)
TMP_LOOKAHEAD = 0
U_ROUTE = "act2"      # ph evac on ACT (own port), DVE muls at 2x
OUT_BF16 = True
TMP1_POOL_EVERY = 0   # >0: emit tmp1 on gpsimd every Nth half-iteration
REPS_FIRST = True     # emit rep matmuls before mains (unblocks DVE/ACT chain)
B_BEFORE_U = False    # emit stage_b(it-1) before stage_u(it)
TMP_FUSED = False     # tmp0+tmp1 as one broadcast-x DVE op
TMP_TILEWIDE = False  # tmp/tmp2 muls span the whole tile (fewer DVE ops)
TMP_AFTER_U = False   # emit stage_tmp after stage_u (needs TMP_LOOKAHEAD>0)
U_SPLIT = False       # per-i h-copies/u_c1 (shorter chain into u_c1)
UC1_AFTER_TMP = True  # scheduling edge: next tmps before u_c1 on DVE
U_C1_POOL_EVERY = 0   # >0: u_c1 on gpsimd every Nth half-iteration
U_DIRECT_EVERY = 0    # >0: chunk-0 u-mul direct from psum every Nth half

# Diagnostic amputations (WRONG RESULTS - timing only)
DIAG_NO_IDENTS = False   # skip u/tmp2 accumulation matmuls
DIAG_NO_UMUL = False     # skip u muls + hsb copies (idents read stale u)
DIAG_NO_TMP2 = False     # skip tmp2 mul (gpsimd idle)
DIAG_NO_REPS = False     # skip rep matmuls + tmp muls + u chain entirely

_cache = {}


def _build(tc_size=TC, reps=1, inner=1):
    import contextlib
    import concourse.mybir as mybir
    import concourse.tile as tile
    from concourse import bacc

    bf16, f32 = mybir.dt.bfloat16, mybir.dt.float32
    nt = tc_size // TB
    nh = nt * 2

    nc = bacc.Bacc("TRN2", target_bir_lowering=False, debug=False)

    x_d = nc.dram_tensor("x", [M, tc_size, S], bf16, kind="ExternalInput")
    wbd_d = nc.dram_tensor("wbd", [nt, 128, 128], bf16, kind="ExternalInput")
    rep_d = nc.dram_tensor("rep", [128, 128], bf16, kind="ExternalInput")
    idn_d = nc.dram_tensor("idn", [128, 128], bf16, kind="ExternalInput")
    ewin_d = nc.dram_tensor("ewin", [128, 3, S], bf16, kind="ExternalInput")
    ewout_d = nc.dram_tensor("ewout", [128, 2, S], bf16, kind="ExternalInput")
    out_dt = bf16 if OUT_BF16 else f32
    out_d = nc.dram_tensor("out", [M, tc_size, S], out_dt,
                           kind="ExternalOutput")

    def eng(name):
        return getattr(nc, {"scalar": "scalar", "vector": "vector",
                            "gpsimd": "gpsimd"}[name])

    def copy_on(engine_name, dst, src):
        if engine_name == "scalar":
            nc.scalar.copy(dst, src)
        else:
            nc.vector.tensor_copy(dst, src)

    with tile.TileContext(nc) as tc:
        with (
            tc.tile_pool(name="const", bufs=1) as constp,
            tc.tile_pool(name="xp", bufs=BUFS["xp"]) as xp,
            tc.tile_pool(name="tmpp", bufs=BUFS["tmpp"]) as tmpp,
            tc.tile_pool(name="t2p", bufs=BUFS["t2p"]) as t2p,
            tc.tile_pool(name="hp", bufs=BUFS["hp"]) as hp,
            tc.tile_pool(name="up", bufs=BUFS["up"]) as up,
            tc.tile_pool(name="op", bufs=BUFS["op"]) as op,
            tc.tile_pool(name="ps_o", bufs=BUFS["ps_o"], space="PSUM") as ps_o,
            tc.tile_pool(name="ps_h", bufs=BUFS["ps_h"], space="PSUM") as ps_h,
        ):
            # Loads ordered by first use: ewin[:, 0:2] feeds the first tmp
            # ops, then the x tiles (emitted by stage_tmp(0)); everything
            # else arrives behind them so the pipeline ramps fast.
            ewin = constp.tile([128, 3, S], bf16)
            nc.sync.dma_start(ewin[:, 0:2], ewin_d.ap()[:, 0:2])
            rep = constp.tile([128, 128], bf16)
            wbd = constp.tile([128, nt, 128], bf16)
            idn = constp.tile([128, 128], bf16)
            ewout = constp.tile([128, 2, S], bf16)

            def load_late_consts():
                nc.sync.dma_start(rep[:], rep_d.ap())
                nc.sync.dma_start(ewin[:, 2:3], ewin_d.ap()[:, 2:3])
                nc.sync.dma_start(ewout[:], ewout_d.ap())
                nc.sync.dma_start(idn[:], idn_d.ap())
                nc.sync.dma_start(
                    wbd[:], wbd_d.ap().rearrange("t p q -> p t q"))

            xt_of_tile = {}
            tw_tiles = {}  # tile -> (tmp, tmp2) for TMP_TILEWIDE
            tmps = {}   # half -> (tmp, tmp2), produced one iteration early
            state = {}  # half -> dict for stage B
            late_consts_done = []
            pending_uc1 = []  # last u_c1 instr, to order next tmps ahead

            def load_x(ti):
                if ti in xt_of_tile or ti >= nt:
                    return
                xt = xp.tile([128, S], bf16)
                src = x_d.ap()[:, ti * TB:(ti + 1) * TB, :].rearrange(
                    "m tb s -> tb m s"
                )
                nc.sync.dma_start(xt[:], src)
                xt_of_tile[ti] = xt

            def stage_tmp(h):
                """Elementwise pre-multiplies for half h (emitted one
                iteration ahead so DVE never waits on this iteration's PE)."""
                ti, half = divmod(h, 2)
                hs = slice(half * HW, (half + 1) * HW)
                load_x(ti)
                load_x(ti + 1)
                load_x(ti + 2)
                xt = xt_of_tile[ti]
                if TMP_TILEWIDE:
                    if half == 1:
                        tmp, tmp2 = tw_tiles[ti]
                        tmps[h] = dict(tmp=tmp, tmp2=tmp2, off=HW)
                        return
                    tmp = tmpp.tile([128, 2, S], bf16)
                    tmp2 = t2p.tile([128, S], bf16)
                    tmp_instrs = [
                        nc.vector.tensor_mul(tmp[:, 0], xt[:], ewin[:, 0]),
                        nc.vector.tensor_mul(tmp[:, 1], xt[:], ewin[:, 1]),
                    ]
                    if UC1_AFTER_TMP and pending_uc1:
                        from concourse.tile import add_dep_helper
                        uc1 = pending_uc1.pop()
                        for tin in tmp_instrs:
                            add_dep_helper(uc1.ins, tin.ins, sync=False,
                                           reason="order next tmps before u_c1")
                    if not late_consts_done:
                        late_consts_done.append(True)
                        load_late_consts()
                    eng(TMP2_ENGINE).tensor_mul(tmp2[:], xt[:], ewin[:, 2])
                    tw_tiles[ti] = (tmp, tmp2)
                    tmps[h] = dict(tmp=tmp, tmp2=tmp2, off=0)
                    return
                tmp = tmpp.tile([128, 2, HW], bf16)
                tmp_instrs = []
                if DIAG_NO_REPS:
                    if not late_consts_done:
                        late_consts_done.append(True)
                        load_late_consts()
                    nc.gpsimd.memset(tmp[:, :, 0:8], 0.0)
                    tmp2 = t2p.tile([128, HW], bf16)
                    if DIAG_NO_TMP2:
                        nc.gpsimd.memset(tmp2[:, 0:8], 0.0)
                    else:
                        eng(TMP2_ENGINE).tensor_mul(tmp2[:], xt[:, hs],
                                                    ewin[:, 2, hs])
                    tmps[h] = dict(tmp=tmp, tmp2=tmp2, off=0)
                    return
                if TMP_FUSED:
                    xb = xt[:, hs].rearrange(
                        "p (one s) -> p one s", one=1
                    ).broadcast_to((128, 2, HW))
                    nc.vector.tensor_mul(tmp[:], xb, ewin[:, 0:2, hs])
                else:
                    tmp_instrs.append(nc.vector.tensor_mul(
                        tmp[:, 0], xt[:, hs], ewin[:, 0, hs]))
                    tmp1_eng = nc.vector
                    if TMP1_POOL_EVERY and h % TMP1_POOL_EVERY == 0:
                        tmp1_eng = nc.gpsimd
                    ins1 = tmp1_eng.tensor_mul(tmp[:, 1], xt[:, hs],
                                               ewin[:, 1, hs])
                    if tmp1_eng is nc.vector:
                        tmp_instrs.append(ins1)
                if UC1_AFTER_TMP and pending_uc1:
                    from concourse.tile import add_dep_helper
                    uc1 = pending_uc1.pop()
                    for tin in tmp_instrs:
                        add_dep_helper(uc1.ins, tin.ins, sync=False,
                                       reason="order next tmps before u_c1")
                if not late_consts_done:
                    late_consts_done.append(True)
                    load_late_consts()
                tmp2 = t2p.tile([128, HW], bf16)
                if DIAG_NO_TMP2:
                    nc.gpsimd.memset(tmp2[:, 0:8], 0.0)
                else:
                    eng(TMP2_ENGINE).tensor_mul(tmp2[:], xt[:, hs],
                                                ewin[:, 2, hs])
                tmps[h] = dict(tmp=tmp, tmp2=tmp2, off=0)

            def stage_mm(h):
                ti, half = divmod(h, 2)
                xt = xt_of_tile[ti]
                tdict = tmps.pop(h)
                tmp, tmp2, toff = tdict["tmp"], tdict["tmp2"], tdict["off"]

                main_stop = DIAG_NO_IDENTS

                def emit_mains():
                    if PO_MERGED:
                        po = ps_o.tile([128, 2, SC], f32)
                        po_cs = [po[:, c] for c in range(2)]
                        st_extra = {"po": po}
                    else:
                        po_cs, st_extra = [], {}
                    for c in range(2):
                        cs = slice(half * HW + c * SC,
                                   half * HW + (c + 1) * SC)
                        if PO_MERGED:
                            po_c = po_cs[c]
                        else:
                            po_c = ps_o.tile([128, SC], f32)
                            po_cs.append(po_c)
                        nc.tensor.matmul(po_c[:] if not PO_MERGED else po_c,
                                         wbd[:, ti], xt[:, cs],
                                         start=True, stop=main_stop)
                    return po_cs, st_extra

                def emit_reps():
                    if DIAG_NO_REPS:
                        return None
                    if U_ROUTE == "act4":
                        phs = []
                        for c in range(2):
                            for i in range(2):
                                ph_ci = ps_h.tile([128, SC], f32)
                                nc.tensor.matmul(
                                    ph_ci[:], rep[:],
                                    tmp[:, i, toff + c * SC:toff + (c + 1) * SC],
                                    start=True, stop=True)
                                phs.append(ph_ci)
                        return phs
                    if U_ROUTE == "merged":
                        ph = ps_h.tile([128, 2, 2, SC], f32)
                        for c in range(2):
                            for i in range(2):
                                nc.tensor.matmul(
                                    ph[:, c, i], rep[:],
                                    tmp[:, i, toff + c * SC:toff + (c + 1) * SC],
                                    start=True, stop=True)
                        return ph
                    phs = []
                    for c in range(2):
                        ph = ps_h.tile([128, 2, SC], f32)
                        for i in range(2):
                            nc.tensor.matmul(
                                ph[:, i], rep[:],
                                tmp[:, i, toff + c * SC:toff + (c + 1) * SC],
                                start=True, stop=True)
                        phs.append(ph)
                    return phs

                if REPS_FIRST:
                    phs = emit_reps()
                    po_cs, st_extra = emit_mains()
                else:
                    po_cs, st_extra = emit_mains()
                    phs = emit_reps()
                state[h] = dict(ti=ti, half=half, po_cs=po_cs, phs=phs,
                                tmp2=tmp2, toff=toff, **st_extra)

            def stage_u(h):
                st = state[h]
                half, phs = st["half"], st.pop("phs")
                u = up.tile([128, 2, HW], bf16)
                if DIAG_NO_UMUL or DIAG_NO_REPS:
                    nc.gpsimd.memset(u[:, :, 0:8], 0.0)
                    st["u"] = u
                    return
                hs = slice(half * HW, (half + 1) * HW)
                route = U_ROUTE
                if route == "alternate":
                    # balance DVE (1x psum TT) vs ACT (evac copy) load
                    route = "direct0" if h % 2 == 0 else "act"
                if route == "merged":
                    # single 4-bank ph tile; one ACT evac + one DVE mul
                    ph = phs
                    hsb = hp.tile([128, 2, 2, SC], bf16)
                    nc.scalar.copy(hsb[:], ph[:])
                    u_ap = u[:].rearrange("p i (c s) -> p c i s", c=2)
                    ew_ap = ewout[:, :, hs].rearrange(
                        "p i (c s) -> p c i s", c=2)
                    nc.vector.tensor_mul(u_ap, hsb[:], ew_ap)
                    st["u"] = u
                    return
                if route == "act4":
                    # per-(chunk,rank) 1-bank ph tiles: finest psum recycle
                    u_cs = []
                    for c in range(2):
                        u_c = up.tile([128, 2, SC], bf16)
                        for i in range(2):
                            cslc = slice(half * HW + c * SC,
                                         half * HW + (c + 1) * SC)
                            hsb_ci = hp.tile([128, SC], bf16)
                            nc.scalar.copy(hsb_ci[:], phs[2 * c + i][:])
                            nc.vector.tensor_mul(u_c[:, i], hsb_ci[:],
                                                 ewout[:, i, cslc])
                        u_cs.append(u_c)
                    st["u_cs"] = u_cs
                    return
                if route == "act2c":
                    # act2 with per-chunk u tiles (contiguous writes,
                    # earlier release into idents)
                    u_cs = []
                    for c in range(2):
                        cslc = slice(half * HW + c * SC, half * HW + (c + 1) * SC)
                        hsb = hp.tile([128, 2, SC], bf16)
                        nc.scalar.copy(hsb[:], phs[c][:])
                        u_c = up.tile([128, 2, SC], bf16)
                        nc.vector.tensor_mul(u_c[:], hsb[:], ewout[:, :, cslc])
                        u_cs.append(u_c)
                    st["u_cs"] = u_cs
                    return
                if route == "act2":
                    # per-chunk ACT evac + DVE 2x mul (GPS idle, ACT loaded)
                    for c in range(2):
                        cslc = slice(half * HW + c * SC, half * HW + (c + 1) * SC)
                        if (U_DIRECT_EVERY and c == 0
                                and h % U_DIRECT_EVERY == 0):
                            # balance: skip ACT evac, mul straight from psum
                            nc.vector.tensor_mul(
                                u[:, :, 0:SC], phs[0][:], ewout[:, :, cslc])
                            continue
                        hsb = hp.tile([128, 2, SC], bf16)
                        nc.scalar.copy(hsb[:], phs[c][:])
                        nc.vector.tensor_mul(
                            u[:, :, c * SC:(c + 1) * SC], hsb[:],
                            ewout[:, :, cslc])
                    st["u"] = u
                    return
                if route == "act1":
                    # per-chunk ACT evacs into ONE hsb tile; single DVE mul
                    hsb = hp.tile([128, 2, 2, SC], bf16)
                    for c in range(2):
                        nc.scalar.copy(hsb[:, c], phs[c][:])
                    u_ap = u[:].rearrange("p i (c s) -> p c i s", c=2)
                    ew_ap = ewout[:, :, hs].rearrange(
                        "p i (c s) -> p c i s", c=2)
                    nc.vector.tensor_mul(u_ap, hsb[:], ew_ap)
                    st["u"] = u
                    return
                if route == "direct0":
                    # chunk 0: multiply straight out of PSUM on DVE (1x);
                    # chunk 1: ACT-copy to SBUF bf16 then DVE TT at 2x.
                    c0 = slice(half * HW, half * HW + SC)
                    c1 = slice(half * HW + SC, half * HW + HW)
                    hsb = hp.tile([128, 2, SC], bf16)
                    if U_SPLIT:
                        # per-i granularity: the first copy starts one
                        # rep-matmul earlier, shortening the chain into u_c1
                        for i in range(2):
                            nc.scalar.copy(hsb[:, i], phs[1][:, i])
                        nc.vector.tensor_mul(u[:, :, 0:SC], phs[0][:],
                                             ewout[:, :, c0])
                        for i in range(2):
                            nc.vector.tensor_mul(u[:, i, SC:HW], hsb[:, i],
                                                 ewout[:, i, c1])
                    else:
                        nc.vector.tensor_mul(u[:, :, 0:SC], phs[0][:],
                                             ewout[:, :, c0])
                        nc.scalar.copy(hsb[:], phs[1][:])
                        uc1_eng = nc.vector
                        if U_C1_POOL_EVERY and h % U_C1_POOL_EVERY == 0:
                            uc1_eng = nc.gpsimd
                        uc1 = uc1_eng.tensor_mul(u[:, :, SC:HW], hsb[:],
                                                 ewout[:, :, c1])
                        if UC1_AFTER_TMP and uc1_eng is nc.vector:
                            pending_uc1.clear()
                            pending_uc1.append(uc1)
                else:  # "act": both chunks evacuated by ACT, one 2x TT
                    hsb = hp.tile([128, 2, HW], bf16)
                    for c in range(2):
                        nc.scalar.copy(hsb[:, :, c * SC:(c + 1) * SC],
                                       phs[c][:])
                    nc.vector.tensor_mul(u[:], hsb[:], ewout[:, :, hs])
                st["u"] = u

            def stage_b(h):
                st = state.pop(h)
                ti, half, po_cs, tmp2 = (st["ti"], st["half"], st["po_cs"],
                                         st["tmp2"])
                u, u_cs = st.get("u"), st.get("u_cs")
                toff = st["toff"]
                ot = op.tile([128, 2, SC], out_dt)
                for c in range(2):
                    csl = slice(c * SC, (c + 1) * SC)
                    po_c = po_cs[c] if PO_MERGED else po_cs[c][:]
                    if not DIAG_NO_IDENTS:
                        u0 = u_cs[c][:, 0] if u_cs is not None else u[:, 0, csl]
                        u1 = u_cs[c][:, 1] if u_cs is not None else u[:, 1, csl]
                        nc.tensor.matmul(po_c, idn[:], u0,
                                         start=False, stop=False)
                        nc.tensor.matmul(po_c, idn[:], u1,
                                         start=False, stop=False)
                        nc.tensor.matmul(po_c, idn[:],
                                         tmp2[:, toff + c * SC:
                                              toff + (c + 1) * SC],
                                         start=False, stop=True)
                    if not PO_MERGED:
                        ename = OUTCOPY_ENGINES[(2 * half + c) % 2]
                        if (OUTCOPY_VECTOR_EVERY and c == 1
                                and h % OUTCOPY_VECTOR_EVERY == 0):
                            ename = "vector"
                        copy_on(ename, ot[:, c], po_c)
                if PO_MERGED:
                    copy_on(OUTCOPY_ENGINES[half % 2], ot[:], st["po"][:])
                dst = out_d.ap()[
                    :, ti * TB:(ti + 1) * TB, half * HW:(half + 1) * HW
                ].rearrange("n tb s -> tb n s")
                nc.sync.dma_start(dst, ot[:].rearrange("p c s -> p (c s)"))

            loop_cm = (tc.For_i(0, reps, 1) if reps > 1
                       else contextlib.nullcontext())
            with loop_cm:
              for _inner in range(inner):
                xt_of_tile.clear()
                tw_tiles.clear()
                for it in range(TMP_LOOKAHEAD):
                    stage_tmp(it)
                for it in range(nh + 1):
                    if not TMP_AFTER_U and it + TMP_LOOKAHEAD < nh:
                        stage_tmp(it + TMP_LOOKAHEAD)
                    if it < nh:
                        stage_mm(it)
                        if not B_BEFORE_U:
                            stage_u(it)
                    if TMP_AFTER_U and it + TMP_LOOKAHEAD < nh:
                        stage_tmp(it + TMP_LOOKAHEAD)
                    if it >= 1:
                        stage_b(it - 1)
                    if B_BEFORE_U and it < nh:
                        stage_u(it)

    nc.compile()
    return nc


def _prep_weights(qw1, qw2, kw1, kw2, qdd, kdd, w, tc_size=TC, ncores=NCORES):
    """Host-side weight folding. Returns per-core wbd + shared tiles."""
    nt = tc_size // TB
    wi = w[0].astype(np.float64) + np.eye(M)
    qw1f, qw2f = qw1[0, :, 0].astype(np.float64), qw2[0, :, 0].astype(np.float64)
    # W_t[m,n] = wi + sum_i qw1[t,i,m] qw2[t,i,n] + diag(qdd[t])
    Wt = wi[None] + np.einsum("tim,tin->tmn", qw1f, qw2f)
    Wt[:, np.arange(M), np.arange(M)] += qdd[0, :, 0].astype(np.float64)
    Wt = Wt.astype(np.float32)

    wbds = []
    for c in range(ncores):
        Wc = Wt[c * tc_size:(c + 1) * tc_size].reshape(nt, TB, M, M)
        wbd = np.zeros((nt, 128, 128), dtype=bf)
        for tb in range(TB):
            wbd[:, tb * M:(tb + 1) * M, tb * M:(tb + 1) * M] = Wc[:, tb].astype(bf)
        wbds.append(wbd)

    rep = np.zeros((128, 128), dtype=bf)
    for tb in range(TB):
        rep[tb * M:(tb + 1) * M, tb * M:(tb + 1) * M] = 1.0
    idn = np.eye(128, dtype=np.float32).astype(bf)

    kw1f = kw1[0, :, 0]  # [S, I, M]
    kw2f = kw2[0, :, 0]
    kddf = kdd[0, :, 0]  # [S, M]
    ewin = np.empty((128, 3, S), dtype=bf)
    ewin[:, 0] = np.tile(kw1f[:, 0, :].T, (TB, 1)).astype(bf)
    ewin[:, 1] = np.tile(kw1f[:, 1, :].T, (TB, 1)).astype(bf)
    ewin[:, 2] = np.tile(kddf.T, (TB, 1)).astype(bf)
    ewout = np.empty((128, 2, S), dtype=bf)
    ewout[:, 0] = np.tile(kw2f[:, 0, :].T, (TB, 1)).astype(bf)
    ewout[:, 1] = np.tile(kw2f[:, 1, :].T, (TB, 1)).astype(bf)
    return wbds, rep, idn, ewin, ewout


def _make_in_maps(inputs, qw1, qw2, kw1, kw2, qdd, kdd, w,
                  tc_size=TC, ncores=NCORES):
    wbds, rep, idn, ewin, ewout = _prep_weights(
        qw1, qw2, kw1, kw2, qdd, kdd, w, tc_size, ncores
    )
    x = np.asarray(inputs)[0]  # [N, T, S] f32
    in_maps = []
    for c in range(ncores):
        xc = np.ascontiguousarray(
            x[:, c * tc_size:(c + 1) * tc_size, :]
        ).astype(bf)
        in_maps.append({
            "x": xc, "wbd": wbds[c], "rep": rep, "idn": idn,
            "ewin": ewin, "ewout": ewout,
        })
    return in_maps


def kernel(inputs, qw1, qw2, kw1, kw2, qdd, kdd, w, trace=False):
    from concourse import bass_utils

    inputs = np.asarray(inputs, dtype=np.float32)
    qw1, qw2 = np.asarray(qw1, np.float32), np.asarray(qw2, np.float32)
    kw1, kw2 = np.asarray(kw1, np.float32), np.asarray(kw2, np.float32)
    qdd, kdd = np.asarray(qdd, np.float32), np.asarray(kdd, np.float32)
    w = np.asarray(w, np.float32)

    if "nc" not in _cache:
        _cache["nc"] = _build()
    nc = _cache["nc"]

    in_maps = _make_in_maps(inputs, qw1, qw2, kw1, kw2, qdd, kdd, w)
    res = bass_utils.run_bass_kernel_spmd(
        nc, in_maps, core_ids=list(range(NCORES)), trace=trace
    )
    outs = [np.asarray(r["out"], dtype=np.float32) for r in res.results]
    out = np.concatenate(outs, axis=1)  # [N,T,S]
    _cache["last_results"] = res
    return out.reshape(B, N, T, S).astype(np.float32)

